# revision 70
# baseline (speedup 1.0000x reference)
"""Trainium2 Bass kernel for nn_FeatureRefinement.

Reference computation (bs=16, vl=1024, ql=64, d=1024):
    corr = einsum('bqd,bvd->bqv', Q, V); scores = softmax(corr, axis=1)
    corr_matrix = einsum('bqv,qd->bvd', scores, cor_w)     # cor_w constant over q
    sentence    = WeightedPool(Q)                           # (bs, d)
    sim         = cosine(V, sentence) + log(video_mask)     # (bs, vl)
    features    = concat([V, sim*sim_w, sentence_bcast, corr_matrix], -1)
    out         = relu(features @ mixer_w + mixer_b)

Algebraic restructuring (exact up to fp rounding):
  - softmax over q sums to 1  =>  corr_matrix[b,v,:] == cor_v_w*cor_q_w  (constant)
  - sim_features @ W2  == sim[b,v] * (sim_w.T @ W2)        (rank-1)
  - pooled_query @ W3  == sentence[b] @ W3                 (rank-1 per batch)
  so   out[b,v,:] = relu(V[b,v,:] @ W1 + sim[b,v]*w2v + bias[b,:])
  All O(n^2) terms are computed on the host in fp32; the device runs ONLY
  the O(n^3) part.

TRANSPOSED layout (trace-driven): the device computes out^T[b, d, v].
With d on partitions, the rank-2 addend decomposes per (d-tile n):
  - bias[b, n*128:+128] is a per-partition [P,1] column -> folded into the
    scalar-engine Relu (activation bias operand), costing zero DVE work;
  - sim[b,v]*w2v: ONE DVE scalar_tensor_tensor per group:
    (simb * w2v_col) + psum, with the PSUM tile as in1.
  The host transposes the output back (cheap numpy).  This removes the v3
  bottleneck where 16 upfront [128,1024] DVE addend instructions (~1.4us
  each) blocked the PSUM-freeing close ops, stalling the PE >3.4us and
  re-throttling the HAM clock gate to half width.

Hardware schedule notes (measured):
  - ~6.7us fixed NEFF preamble before any user instruction, and a long
    semaphore-clear teardown after; first DMA data lands ~3.5us after its
    dma_start reaches the head of a queue.
  - HAM clock gate: evaluates PE duty in ~3.4us quanta -- half width
    until a near-gapless quantum, demoted again by any quantum below
    ~50% duty (one ~2us feed-jitter gap suffices; each demotion costs
    ~2-3us).  The DMA completion timeline has a systematic ~4us HOLE:
    items 1-4 land ~10-11.2us, then nothing until ~14-15.4us, then a
    tight burst.  Each queue's FIRST transfer beats the hole, so the
    host packs chunk-pair 1 (w1-low k1 + vtb0 k1) into one contiguous
    "bundle" tensor riding gpsimd's fast slot: pairs 0 AND 1 are banked
    by ~11.5us, 10 junk matmuls flip the HAM and the real stream starts
    ~11.9us with two full wave rounds (~6.9us) in hand -- enough to
    coast through the hole and into the burst, gap-free to the end.
  - Per-queue ~110-150 GB/s; only ~2 queues' worth of aggregate early
    bandwidth, so gpsimd carries just the small close operands while the
    scalar+sync queues interleave the wave-critical feed (w1 lower
    column-halves + batch-0 V^T chunks, k-pairwise), with the w1 upper
    halves and batch-1 V behind them.  Output stores ride gpsimd except
    the last few, which move to sync/scalar so gpsimd's queue is empty
    long before the end (a gpsimd DRAIN with in-flight transfers costs
    ~2.5us; drained early it costs ~0.1us).
  - Two 8-group chunk-chasing waves over batch 0, then plain h-outer
    groups (8 consecutive matmuls per PSUM bank: per-instruction bank
    alternation costs ~46ns/matmul).  Steady state is PE-bound at 216ns
    per 512-row fp16 matmul; the 256-matmul stream (~55.3us) is the
    fp16 roofline for V @ W1 at 78.6 TF/s.  fp8 cannot beat it: e4m3
    quantization needs >=3 DoubleRow passes to stay under the 2e-2
    error gate, which is 1.5x the fp16 cost.
  - The kernel tail is (last matmul) -> stt -> relu -> store -> queue
    drain -> barrier; the final d-tile runs a narrow-group epilogue
    (512/256/128/128 columns, the last two closed entirely on the DVE
    with stores on idle engines) so only an eighth-width chain and a
    32KB store remain on the critical path.
"""
import sys

sys.path.insert(0, "/opt/trn_rl_repo")

import numpy as np
from contextlib import ExitStack

import concourse.bass as bass
import concourse.tile as tile
from concourse import bacc, mybir
from concourse.bass_utils import run_bass_kernel_spmd


def _install_ntff_shim():
    """This container's antenv lacks axon_hooks; if tracing is requested
    (BASS_TRACE=1), run_bass_kernel_spmd would crash importing it. Provide
    the hook via trn_agent_boot's ctypes helper, and keep the trace
    post-processing local (no bucket uploads)."""
    import types
    try:
        import antenv  # noqa: F401
        import antenv.axon_hooks  # noqa: F401
        return  # already present
    except ImportError:
        pass
    try:
        import trn_agent_boot.trn_boot as _tb
        hook = _tb._ntff_profile_via_ctypes("/opt/axon/libaxon_pjrt.so")
        mod = types.ModuleType("antenv.axon_hooks")
        mod.get_axon_ntff_profile_hook = lambda: hook
        sys.modules["antenv.axon_hooks"] = mod
        from concourse import bass_utils as _bu
        _orig = _bu.upload_artifacts

        def _safe_upload(tmpdir):
            try:
                return _orig(tmpdir)
            except Exception:
                return f"file://{tmpdir}"

        _bu.upload_artifacts = _safe_upload
    except Exception:
        pass


_install_ntff_shim()

F32 = mybir.dt.float32
F16 = mybir.dt.float16
ALU = mybir.AluOpType
AF = mybir.ActivationFunctionType

BS, VL, QL, D = 16, 1024, 64, 1024
NCORES = 8
BPC = BS // NCORES          # batches per core
KC = D // 128               # contraction chunks
ND = D // 128               # output d-tiles (psum partition tiles)
NEG_INF = -1e30

N_WARM = 10                 # junk matmuls to warm the PE HAM gate
WAVE_N = 4                  # batch-0 d-tiles in the chunk-chasing wave


def _build_program():
    nc = bacc.Bacc("TRN2", target_bir_lowering=False, debug=False,
                   num_devices=NCORES)

    w1_d = nc.dram_tensor("w1", [KC, 128, D], F16, kind="ExternalInput").ap()
    vtb_d = nc.dram_tensor("vtb", [BPC, KC, 128, VL], F16,
                           kind="ExternalInput").ap()
    simb_d = nc.dram_tensor("simb", [BPC, 128, VL], F16,
                            kind="ExternalInput").ap()
    # chunk-pair-1 bundle: [w1 k=1 lower half | vtb0 k=1] packed so ONE
    # DMA (gpsimd's fast first slot) delivers the whole second wave round
    bun_d = nc.dram_tensor("bun", [128, 1536], F16, kind="ExternalInput").ap()
    # cols[:, 0:8] = w2v columns; cols[:, 8+b*8+n] = bias[b] column n
    cols_d = nc.dram_tensor("cols", [128, ND + BPC * ND], F32,
                            kind="ExternalInput").ap()
    out_d = nc.dram_tensor("out", [BPC, D, VL], F16, kind="ExternalOutput").ap()

    with tile.TileContext(nc) as tc, ExitStack() as ctx:
        singles = ctx.enter_context(tc.tile_pool(name="singles", bufs=1))
        vtp = ctx.enter_context(tc.tile_pool(name="vtp", bufs=BPC * KC))
        opool = ctx.enter_context(tc.tile_pool(name="opool", bufs=3))
        opool1 = ctx.enter_context(tc.tile_pool(name="opool1", bufs=4))
        tpool = ctx.enter_context(tc.tile_pool(name="tpool", bufs=4))
        psOut = ctx.enter_context(tc.tile_pool(name="psOut", bufs=8,
                                               space="PSUM"))

        w1_sb = singles.tile([128, KC, D], F16)
        vt0 = {k: vtp.tile([128, VL], F16, tag="vt", name=f"vt0_{k}")
               for k in range(KC) if k != 1}
        vt1 = singles.tile([128, KC, VL], F16)
        simb = singles.tile([128, BPC, VL], F16)
        cols = singles.tile([128, ND + BPC * ND], F32)
        bun = singles.tile([128, 1536], F16)

        def vtile(b, k):
            if b == 0:
                return bun[:, 512:1536] if k == 1 else vt0[k]
            return vt1[:, k, :]

        def w1slice(n, k):
            if k == 1 and n < WAVE_N:
                return bun[:, n * 128:(n + 1) * 128]
            return w1_sb[:, k, n * 128:(n + 1) * 128]

        # memset on gpsimd (first instruction after its preamble exit);
        # the junk matmuls depend only on this.  (The preamble's
        # all-engine rendezvous dominates start jitter, not this engine
        # choice; a writer is mandatory -- the tile framework rejects
        # read-without-write tiles.)
        warm16 = singles.tile([128, 512], F16)
        nc.gpsimd.memset(warm16, 0.0)

        # Wave-critical feed.  Wave-1 (b0, n0-3) needs only the LOWER
        # column half of each w1 chunk, so the early feed is w1-low (1MB)
        # + vtb0 (2MB) interleaved k-pairwise across scalar+sync: chunk k
        # completes at ~10.3 + 1.3k us (vs 1.9k with whole chunks).  The
        # upper w1 halves (wave-2) and batch-1 V stream behind as bulk
        # DMAs.  gpsimd carries ONLY the small close operands early: the
        # early window sustains ~2 queues of aggregate bandwidth, and any
        # extra gpsimd traffic slows the feed, opens >3.4us PE gaps, and
        # re-throttles the HAM clock.
        for k in range(KC):
            if k == 1:
                continue  # chunk-pair 1 arrives via the gpsimd bundle
            e0, e1 = (nc.scalar, nc.sync) if k % 2 == 0 else (nc.sync,
                                                              nc.scalar)
            e0.dma_start(out=w1_sb[:, k, 0:512], in_=w1_d[k, :, 0:512])
            e1.dma_start(out=vt0[k], in_=vtb_d[0, k])
        for q in range(4):
            ksl = slice(q * 2, q * 2 + 2)
            nc.scalar.dma_start(
                out=w1_sb[:, ksl, 512:1024],
                in_=w1_d[ksl, :, 512:1024].rearrange("k p n -> p k n"))
        for half in range(2):
            ksl = slice(half * 4, half * 4 + 4)
            nc.sync.dma_start(out=vt1[:, ksl, :],
                              in_=vtb_d[1, ksl].rearrange("k p v -> p k v"))
        # gpsimd: the chunk-pair-1 bundle rides the queue's fast first
        # slot (~11.3us, before the completion hole), then the small close
        # operands; later it carries the output stores.
        nc.gpsimd.dma_start(out=bun, in_=bun_d)
        nc.gpsimd.dma_start(out=cols, in_=cols_d)
        nc.gpsimd.dma_start(out=simb, in_=simb_d.rearrange("b p v -> p b v"))

        # ================= PE HAM warmup ===========================
        # 14 junk matmuls (~6us at the half-width clock): the HAM gate
        # flips to full width mid-warmup, and by the time the real stream
        # starts (~13.5us) the first 2-3 feed chunk-pairs are banked, so
        # DMA jitter can no longer open a >3.4us PE gap (which would
        # re-throttle the clock for a costly 3.4us half-width window).
        for r in range(N_WARM):
            warm_ps = psOut.tile([128, 512], F32, tag="o_ps", name=f"warm{r}")
            nc.tensor.matmul(warm_ps, warm16[:, 0:128], warm16,
                             start=True, stop=True)

        # ================= matmul stream ===========================
        ps_of = {}
        out_sb = {}

        def open_group(b, n, h):
            ps_of[(b, n, h)] = psOut.tile([128, 512], F32, tag="o_ps",
                                          name=f"ps{b}_{n}_{h}")

        def mm(b, n, h, k):
            nc.tensor.matmul(ps_of[(b, n, h)], w1slice(n, k),
                             vtile(b, k)[:, h * 512:(h + 1) * 512],
                             start=(k == 0), stop=(k == KC - 1))

        # out tiles: quads [128, 4, VL] stored with ONE DMA per 4 d-tiles
        # (fewer completion semaphores -> shorter NEFF teardown); the last
        # three d-tiles store singly, the final one split for a short tail.
        def out_slot(b, n):
            q = n // 4
            if (b, q) == (1, 1):
                if (b, n) not in out_sb:
                    out_sb[(b, n)] = opool1.tile([128, 1, VL], F16, tag="o1",
                                                 name=f"o16_{b}_{n}")
                return out_sb[(b, n)][:, 0, :]
            if (b, q) not in out_sb:
                out_sb[(b, q)] = opool.tile([128, 4, VL], F16, tag="o16",
                                            name=f"o16_{b}_{q}")
            return out_sb[(b, q)][:, n % 4, :]

        def close_group(b, n, h):
            ps = ps_of.pop((b, n, h))
            ot = out_slot(b, n)
            sl = slice(h * 512, (h + 1) * 512)
            tmp = tpool.tile([128, 512], F16, tag="tmp", name=f"tmp{b}{n}{h}")
            # tmp = sim[b,v]*w2v[n*128+p] + psum
            nc.vector.scalar_tensor_tensor(
                out=tmp, in0=simb[:, b, sl], scalar=cols[:, n:n + 1],
                in1=ps, op0=ALU.mult, op1=ALU.add)
            # out = relu(tmp + bias[b, n*128+p]) on the ACT engine: the
            # per-partition activation bias makes it free, and splitting
            # the close across DVE+ACT halves the PSUM-free latency at the
            # wave transitions (PE waits on the stt to reuse a bank).
            bc = ND + b * ND + n
            nc.scalar.activation(ot[:, sl], tmp, AF.Relu,
                                 bias=cols[:, bc:bc + 1])
            if h == 1:
                if n % 4 == 3 and not (b == 1 and n >= 4):
                    q = n // 4
                    ot4 = out_sb.pop((b, q))
                    dst = out_d[b, q * 512:(q + 1) * 512, :].rearrange(
                        "(j p) v -> p j v", p=128)
                    nc.gpsimd.dma_start(out=dst, in_=ot4)
                elif b == 1 and 4 <= n < ND - 1:
                    # n4/n5 ride sync/scalar: keeps those queues warm for
                    # the epilogue pieces, and empties gpsimd's queue well
                    # before the end (its DRAIN costs ~2.5us if its last
                    # transfer lands near the final barrier)
                    ft = out_sb.pop((b, n))
                    dst = out_d[b, n * 128:(n + 1) * 128, :]
                    eng = {4: nc.sync, 5: nc.scalar, 6: nc.gpsimd}[n]
                    eng.dma_start(out=dst, in_=ft[:, 0, :])

        # Two 7-group waves over batch 0, k-PAIR rounds chasing chunk
        # arrivals with same-bank runs of 2 (bank alternation costs
        # ~46ns/matmul).  Each group closes inline right after its last
        # matmul so the DVE drains PSUM while the PE continues; a junk
        # keep-alive at every round boundary bounds any feed-jitter stall
        # below the HAM re-throttle threshold.
        def wave(groups):
            for g in groups:
                open_group(*g)
            for kp in range(0, KC, 2):
                last = kp == KC - 2
                for g in groups:
                    mm(*g, kp)
                    mm(*g, kp + 1)
                    if last:
                        close_group(*g)

        wave([(0, n, h) for n in range(WAVE_N) for h in range(2)])
        wave([(0, n, h) for n in range(WAVE_N, ND) for h in range(2)])
        # batch 1: everything resident; h-outer keeps 8 consecutive
        # matmuls on one PSUM bank.  The final tile gets its own epilogue.
        for b, n in [(1, n) for n in range(ND - 1)]:
            for h in range(2):
                open_group(b, n, h)
            for h in range(2):
                for k in range(KC):
                    mm(b, n, h, k)
            for h in range(2):
                close_group(b, n, h)

        # ===== final-tile epilogue: narrow trailing groups ============
        # The kernel tail is (last matmul) -> stt -> relu -> store ->
        # queue-drain barrier.  Splitting the final tile's h1 into two
        # 256-column PSUM groups pipelines the first chains under the
        # remaining matmuls and leaves only a quarter-width stt/relu and a
        # 64KB store on the critical path.
        fb, fn = 1, ND - 1
        fdst = out_d[fb, fn * 128:(fn + 1) * 128, :]
        ft = opool1.tile([128, 1, VL], F16, tag="o1", name="o16_final")[:, 0, :]
        fbc = ND + fb * ND + fn

        def fin_close(lo, w, eng, on_act):
            ps = ps_of.pop(("f", lo))
            tmp = tpool.tile([128, w], F16, tag="tmp", name=f"tmpF{lo}")
            nc.vector.scalar_tensor_tensor(
                out=tmp, in0=simb[:, fb, lo:lo + w], scalar=cols[:, fn:fn + 1],
                in1=ps, op0=ALU.mult, op1=ALU.add)
            if on_act:
                nc.scalar.activation(ft[:, lo:lo + w], tmp, AF.Relu,
                                     bias=cols[:, fbc:fbc + 1])
            else:
                # relu on DVE keeps the tail chain off the scalar engine
                # (its ACT + store-issue serialization cost ~0.9us here)
                nc.vector.tensor_scalar(out=ft[:, lo:lo + w], in0=tmp,
                                        scalar1=cols[:, fbc:fbc + 1],
                                        scalar2=0.0, op0=ALU.add,
                                        op1=ALU.max)
            eng.dma_start(out=fdst[:, lo:lo + w], in_=ft[:, lo:lo + w])

        for lo, w, eng, on_act in (
                (0, 512, nc.sync, True), (512, 256, nc.scalar, True),
                (768, 128, nc.sync, False), (896, 128, nc.scalar, False)):
            ps_of[("f", lo)] = psOut.tile([128, w], F32, tag="o_ps",
                                          name=f"psF{lo}")
            for k in range(KC):
                nc.tensor.matmul(ps_of[("f", lo)],
                                 w1_sb[:, k, fn * 128:(fn + 1) * 128],
                                 vtile(fb, k)[:, lo:lo + w],
                                 start=(k == 0), stop=(k == KC - 1))
            fin_close(lo, w, eng, on_act)

    nc.compile()
    return nc


_NC = None
_LAST_RESULTS = None


def _get_program():
    global _NC
    if _NC is None:
        _NC = _build_program()
    return _NC


def kernel(video_features, query_features, video_mask, query_mask,
           sim_w, cor_v_w, cor_q_w, pool_w, mixer_w, mixer_b):
    V = np.asarray(video_features, dtype=np.float32)
    Q = np.asarray(query_features, dtype=np.float32)
    vmask = np.asarray(video_mask, dtype=np.float32)
    qmask = np.asarray(query_mask, dtype=np.float32)
    sim_w = np.asarray(sim_w, dtype=np.float32)
    cor_v_w = np.asarray(cor_v_w, dtype=np.float32)
    cor_q_w = np.asarray(cor_q_w, dtype=np.float32)
    pool_w = np.asarray(pool_w, dtype=np.float32)
    mixer_w = np.asarray(mixer_w, dtype=np.float32)
    mixer_b = np.asarray(mixer_b, dtype=np.float32)

    W1 = mixer_w[0:D]
    W2 = mixer_w[D:2 * D]
    W3 = mixer_w[2 * D:3 * D]
    W4 = mixer_w[3 * D:4 * D]

    # ---- host-side O(n^2) math in fp32 (exact reference semantics) ----
    alpha = Q @ pool_w[:, 0] + (1.0 - qmask) * NEG_INF          # (bs, ql)
    alpha = alpha - alpha.max(axis=1, keepdims=True)
    ea = np.exp(alpha)
    alphas = ea / ea.sum(axis=1, keepdims=True)
    sentence = np.einsum('bqd,bq->bd', Q, alphas)               # (bs, d)
    dot = np.einsum('bvd,bd->bv', V, sentence)                  # (bs, vl)
    vn = np.maximum(np.linalg.norm(V, axis=-1), 1e-8)
    sn = np.maximum(np.linalg.norm(sentence, axis=-1), 1e-8)
    sim = dot / (vn * sn[:, None]) + np.log(vmask + 1e-45)      # (bs, vl)
    w2v = sim_w[:, 0] @ W2                                      # (d,)
    cor_vec = cor_v_w[0] * cor_q_w[0, 0]
    bias = sentence @ W3 + (cor_vec @ W4 + mixer_b)             # (bs, d)

    # ---- device layouts ----
    W1k = np.ascontiguousarray(W1.reshape(KC, 128, D)).astype(np.float16)
    v16 = V.astype(np.float16)
    sim16 = sim.astype(np.float16)
    w2vc = np.ascontiguousarray(w2v.reshape(ND, 128).T).astype(np.float32)

    nc = _get_program()
    in_maps = []
    for c in range(NCORES):
        sl = slice(c * BPC, (c + 1) * BPC)
        # vtb[b,k,p,v] = V[b, v, k*128+p]
        vtb = np.ascontiguousarray(
            v16[sl].transpose(0, 2, 1)).reshape(BPC, KC, 128, VL)
        simb = np.ascontiguousarray(
            np.broadcast_to(sim16[sl][:, None, :], (BPC, 128, VL)))
        cols = np.empty((128, ND + BPC * ND), dtype=np.float32)
        cols[:, 0:ND] = w2vc
        cols[:, ND:] = bias[sl].reshape(BPC * ND, 128).T
        bun = np.concatenate([W1k[1, :, 0:512], vtb[0, 1]], axis=1)
        in_maps.append({"w1": W1k, "vtb": vtb, "simb": simb, "cols": cols,
                        "bun": np.ascontiguousarray(bun)})
    res = run_bass_kernel_spmd(nc, in_maps, core_ids=list(range(NCORES)))
    global _LAST_RESULTS
    _LAST_RESULTS = res
    outT = np.concatenate([res.results[c]["out"] for c in range(NCORES)],
                          axis=0)                               # (bs, d, vl)
    return np.ascontiguousarray(outT.swapaxes(1, 2)).astype(np.float32)
